# revision 1
# baseline (speedup 1.0000x reference)
"""Trainium2 Bass kernel for nn_DensePoseV1ConvXGNInsHead:
2x (conv3x3 64->64 -> per-instance BN -> ReLU) on [8,64,256,256],
data-parallel one image per NeuronCore across 8 cores.

Self-contained: only imports the system concourse stack from /opt/trn_rl_repo.
"""
import os
import sys
import types

sys.path.insert(0, "/opt/trn_rl_repo")

import numpy as np

import concourse.bass as bass
import concourse.tile as tile
from concourse import mybir
from concourse.vector_clock import ScopedClock

f16 = mybir.dt.float16
f32 = mybir.dt.float32
i16 = mybir.dt.int16
ALU = mybir.AluOpType

C = 64          # channels
W = 256         # image width
PITCH = 272     # padded row pitch (16 left pad + 256 data; borrows next row's pad)
LP = 16         # left pad elements
R = 4           # conv rows per block (per half)
EPS = 1e-5
KDEBUG = False

# ---------------------------------------------------------------------------
# walrus workaround: split the Tile exit-drain's sem waits (installed walrus
# rejects instructions with >2 sync waits)
# ---------------------------------------------------------------------------
_patched = False


def _install_tile_patch():
    global _patched
    if _patched:
        return
    _patched = True

    def _drain_and_barrier(self, tick_clock, wait_clock):
        nc = self.nc
        drain_inst = nc.sync.drain()
        wait_clock.add_sem_waits(
            drain_inst.ins, ScopedClock({None: tick_clock.global_clock})
        )
        si = drain_inst.ins.sync_info
        waits = list(si.on_wait or [])
        if len(waits) > 1:
            si.on_wait = waits[:1]
            for i in range(1, len(waits)):
                nop = nc.sync.nop()
                nop.ins.sync_info = mybir.SyncInfo(
                    on_wait=waits[i : i + 1], on_update=[]
                )
        nc.all_engine_barrier()
        popped = nc._tile_sem_poison_stack.pop()
        assert popped is self._sem_poison
        nc.clear_and_free_semaphores(list(self.sems.allocated().values()))
        nc.all_engine_barrier()

    tile.TileContext._drain_and_barrier = _drain_and_barrier


# ---------------------------------------------------------------------------
# NTFF profiling shim (antenv.axon_hooks is absent in this image)
# ---------------------------------------------------------------------------
def _install_ntff_shim():
    if "antenv.axon_hooks" in sys.modules:
        return
    mod = types.ModuleType("antenv.axon_hooks")
    state = {"hook": None}
    mod.set_axon_ntff_profile_hook = lambda h: state.__setitem__("hook", h)
    mod.get_axon_ntff_profile_hook = lambda: state["hook"]
    sys.modules["antenv.axon_hooks"] = mod
    try:
        import antenv

        antenv.axon_hooks = mod
    except ImportError:
        pass
    try:
        from trn_agent_boot.trn_boot import _ntff_profile_via_ctypes

        h = _ntff_profile_via_ctypes("/opt/axon/libaxon_pjrt.so")
        mod.set_axon_ntff_profile_hook(h)
    except Exception:
        pass


def yoff(slot):
    return slot * PITCH + LP


def _ap(base_ap, offset_elems, dims):
    """Build a sub-AP of base_ap at +offset (elements), with given free dims.

    base_ap must be a plain [P, F] tile AP; dims is a list of [step, count]
    free dims; partition dim is preserved."""
    return bass.AP(
        tensor=base_ap.tensor,
        offset=base_ap.offset + offset_elems,
        ap=[base_ap.ap[0]] + dims,
    )


def _dbg_dump(nc, ysb, dst, NCH, HH, nm, outp):
    H = HH * 2
    for g in range(NCH):
        stg = outp.tile([128, 1024], f32, tag="st", name=f"dbg{nm}_{g}")
        yv2 = _ap(ysb[:], yoff(4 * g + 1), [[PITCH, 4], [1, W]])
        nc.vector.tensor_copy(stg[:], yv2)
        nc.sync.dma_start(
            out=bass.AP(
                tensor=dst[:].tensor,
                offset=4 * g * W,
                ap=[[HH * W, 2], [H * W, 64], [W, 4], [1, W]],
            ),
            in_=stg[:],
        )


def emit(nc, H):
    """Emit the full 2-layer kernel for an HxW image (H=256 in production)."""
    HH = H // 2
    NB = HH // R            # conv blocks per layer
    NCH = HH // 4           # normalize chunks (4 rows each) per layer
    NST = HH * 2            # stats chunks (128 px each) per layer
    assert HH % R == 0 and HH % 4 == 0

    xh = nc.declare_dram_parameter("xh", [C, H * W], f16, isOutput=False)
    idsf = nc.declare_dram_parameter("idsf", [H * W], f16, isOutput=False)
    rcnt = nc.declare_dram_parameter("rcnt", [9], f32, isOutput=False)
    w0d = nc.declare_dram_parameter("w0d", [128, 9, 128], f16, isOutput=False)
    w1d = nc.declare_dram_parameter("w1d", [128, 9, 128], f16, isOutput=False)
    id128 = nc.declare_dram_parameter("id128", [128, 128], f16, isOutput=False)
    g0 = nc.declare_dram_parameter("g0", [C], f32, isOutput=False)
    b0 = nc.declare_dram_parameter("b0", [C], f32, isOutput=False)
    g1 = nc.declare_dram_parameter("g1", [C], f32, isOutput=False)
    b1 = nc.declare_dram_parameter("b1", [C], f32, isOutput=False)
    out = nc.declare_dram_parameter("out", [C, H * W], f32, isOutput=True)
    if KDEBUG:
        dbg_y1 = nc.declare_dram_parameter("dbg_y1", [C, H * W], f32, isOutput=True)
        dbg_y1n = nc.declare_dram_parameter("dbg_y1n", [C, H * W], f32, isOutput=True)
        dbg_ssb = nc.declare_dram_parameter("dbg_ssb", [2, 18, 256], f32, isOutput=True)
        dbg_tab = nc.declare_dram_parameter("dbg_tab", [2, 18, 128], f32, isOutput=True)
        dbg_mask = nc.declare_dram_parameter("dbg_mask", [128, NST, 18], f32, isOutput=True)

    with tile.TileContext(nc) as tc:
        import contextlib

        with contextlib.ExitStack() as ctx:
            const = ctx.enter_context(tc.tile_pool(name="const", bufs=1))
            xbp = ctx.enter_context(tc.tile_pool(name="xbp", bufs=1))
            stripp = ctx.enter_context(tc.tile_pool(name="stripp", bufs=3))
            normp = ctx.enter_context(tc.tile_pool(name="normp", bufs=3))
            sop = ctx.enter_context(tc.tile_pool(name="sop", bufs=3))
            outp = ctx.enter_context(tc.tile_pool(name="outp", bufs=3))
            smallp = ctx.enter_context(tc.tile_pool(name="smallp", bufs=2))
            psc = ctx.enter_context(tc.tile_pool(name="psc", bufs=2, space="PSUM"))
            pss = ctx.enter_context(tc.tile_pool(name="pss", bufs=1, space="PSUM"))
            pse = ctx.enter_context(tc.tile_pool(name="pse", bufs=3, space="PSUM"))

            # ---- persistent buffers
            ysb = const.tile([128, (HH + 2) * PITCH + LP], f16)
            nc.vector.memset(ysb[:], 0.0)
            xb0 = xbp.tile([128, (R + 2) * PITCH + LP], f16, tag="xb0")
            xb1 = xbp.tile([128, (R + 2) * PITCH + LP], f16, tag="xb1")
            nc.vector.memset(xb0[:], 0.0)
            nc.vector.memset(xb1[:], 0.0)
            xbs = [xb0, xb1]

            # ---- ids in pixel-major-chunk layout + one-hot masks
            idp = const.tile([128, 2, NST], f16)
            for h in (0, 1):
                src = bass.AP(
                    tensor=idsf[:].tensor,
                    offset=h * HH * W,
                    ap=[[1, 128], [W, HH], [128, 2]],
                )
                nc.sync.dma_start(out=idp[:, h, :], in_=src)
            ktile = const.tile([128, 9], f16)
            nc.gpsimd.iota(
                ktile[:], pattern=[[1, 9]], base=0, channel_multiplier=0,
                allow_small_or_imprecise_dtypes=True,
            )
            nc.vector.memset(ktile[:, 8:9], -1.0)
            maskpm = const.tile([128, NST, 18], f16)
            for h in (0, 1):
                o = maskpm[:]
                outv = _ap(o, 9 * h, [[18, NST], [1, 9]])
                in0 = _ap(idp[:], h * NST, [[1, NST], [0, 9]])
                in1 = _ap(ktile[:], 0, [[0, NST], [1, 9]])
                nc.vector.tensor_tensor(outv, in0, in1, ALU.is_equal)

            # ---- small constants
            id128sb = const.tile([128, 128], f16)
            nc.sync.dma_start(out=id128sb[:], in_=id128[:])
            zf16 = const.tile([128, 1], f16)
            nc.vector.memset(zf16[:], 0.0)
            rcsb = const.tile([9, 1], f32)
            nc.sync.dma_start(out=rcsb[:], in_=rcnt[:].rearrange("(a b) -> a b", b=1))
            ones1 = const.tile([1, 64], f32)
            zeros1 = const.tile([1, 64], f32)
            nc.vector.memset(ones1[:], 1.0)
            nc.vector.memset(zeros1[:], 0.0)
            epsap = const.tile([9, 1], f32)
            nc.vector.memset(epsap[:], EPS)
            # kvec18: [0..7, -1] twice (per-partition compare constants)
            kvec18 = const.tile([18, 1], f32)
            nc.gpsimd.iota(kvec18[0:9, :], pattern=[[0, 1]], base=0,
                           channel_multiplier=1, allow_small_or_imprecise_dtypes=True)
            neg1 = const.tile([1, 1], f32)
            nc.vector.memset(neg1[:], -1.0)
            nc.sync.dma_start(out=kvec18[8:9, :], in_=neg1[:])
            nc.sync.dma_start(out=kvec18[9:18, :], in_=kvec18[0:9, :])
            # segment-major one-hot masks [18, HH*W] fp16 (rows 0:9 half A, 9:18 half B)
            HW2 = HH * W
            ms2 = const.tile([18, HW2], f16)
            MCH = min(4096, HW2)
            for mc in range(HW2 // MCH):
                idsm = smallp.tile([18, MCH], f16, tag="idsm", name=f"idsm{mc}")
                nc.sync.dma_start(
                    out=idsm[:],
                    in_=bass.AP(
                        tensor=idsf[:].tensor,
                        offset=mc * MCH,
                        ap=[[HH * W, 2], [0, 9], [1, MCH]],
                    ),
                )
                nc.vector.tensor_scalar(
                    out=ms2[:, mc * MCH : (mc + 1) * MCH], in0=idsm[:],
                    scalar1=kvec18[:], scalar2=None, op0=ALU.is_equal,
                )
            gam = []
            bet = []
            for gg, bb in ((g0, b0), (g1, b1)):
                gt = const.tile([9, 64], f32, tag="gam")
                bt = const.tile([9, 64], f32, tag="bet")
                nc.sync.dma_start(out=gt[:], in_=gg[:].partition_broadcast(9))
                nc.sync.dma_start(out=bt[:], in_=bb[:].partition_broadcast(9))
                gam.append(gt)
                bet.append(bt)
            wts = []
            for wd in (w0d, w1d):
                wt = const.tile([128, 9, 128], f16, tag="wt")
                nc.sync.dma_start(out=wt[:], in_=wd[:])
                wts.append(wt)

            for L in (0, 1):
                wt = wts[L]
                slot0 = 1 if L == 0 else 0   # y row r lives at slot r+slot0
                stats = pss.tile([18, 256], f32, tag="stats")
                strip_tiles = []

                # ================= conv + stats =================
                ci_count = 0
                for b in range(NB):
                    r0 = b * R
                    if L == 0:
                        xb = xbs[b % 2]
                        # load rows r0-1 .. r0+R into slots 0..R+1 (per half)
                        if b == 0:
                            nc.vector.memset(xb[0:64, 0:PITCH], 0.0)
                        if b == NB - 1:
                            nc.vector.memset(
                                xb[64:128, (R + 1) * PITCH : (R + 2) * PITCH], 0.0
                            )
                        lo_a = r0 - 1
                        s_a = 0
                        if b == 0:
                            lo_a, s_a = 0, 1
                        n_a = r0 + R - lo_a + 1
                        nc.sync.dma_start(
                            out=_ap(xb[0:64, :], yoff(s_a), [[PITCH, n_a], [1, W]]),
                            in_=bass.AP(
                                tensor=xh[:].tensor,
                                offset=lo_a * W,
                                ap=[[H * W, 64], [W, n_a], [1, W]],
                            ),
                        )
                        hb_lo = HH + r0 - 1
                        n_b = R + 2 if b < NB - 1 else R + 1
                        nc.sync.dma_start(
                            out=_ap(xb[64:128, :], yoff(0), [[PITCH, n_b], [1, W]]),
                            in_=bass.AP(
                                tensor=xh[:].tensor,
                                offset=hb_lo * W,
                                ap=[[H * W, 64], [W, n_b], [1, W]],
                            ),
                        )
                        src_t = xb
                        loc = lambda rr, dy: (rr - r0 + 1 + dy)  # slot in xb
                    else:
                        src_t = ysb
                        loc = lambda rr, dy: (rr + dy + 1)       # y1 slot

                    # conv: tap-outer over R//2 psum chunks
                    pts = [
                        psc.tile([128, 1024], f32, tag="cps", name=f"cps_{L}_{b}_{i}")
                        for i in range(R // 2)
                    ]
                    for t in range(9):
                        dy, dx = t // 3 - 1, t % 3 - 1
                        for cp in range(R // 2):
                            rr = r0 + 2 * cp
                            off = yoff(loc(rr, dy)) + dx
                            rhsA = _ap(src_t[0:64, :], off, [[PITCH, 2], [1, W]])
                            rhsB = _ap(src_t[64:128, :], off, [[PITCH, 2], [1, W]])
                            nc.tensor.matmul(
                                pts[cp][0:64, 0:512], wt[0:64, t, 0:64], rhsA,
                                start=(t == 0), stop=(t == 8), tile_position=(0, 0),
                            )
                            nc.tensor.matmul(
                                pts[cp][64:128, 512:1024], wt[64:128, t, 64:128], rhsB,
                                start=(t == 0), stop=(t == 8), tile_position=(64, 64),
                            )
                    for cp in range(R // 2):
                        rr = r0 + 2 * cp
                        dstA = _ap(ysb[0:64, :], yoff(rr + slot0), [[PITCH, 2], [1, W]])
                        dstB = _ap(ysb[64:128, :], yoff(rr + slot0), [[PITCH, 2], [1, W]])
                        nc.scalar.copy(out=dstA, in_=pts[cp][0:64, 0:512])
                        nc.scalar.copy(out=dstB, in_=pts[cp][64:128, 512:1024])

                    # stats for this block: R rows x 2 spans = 2R chunks,
                    # transposed on the PE into a recycled conv-psum tile
                    pts2 = psc.tile([128, 1024], f16, tag="cps", name=f"tp_{L}_{b}")
                    for j in range(2 * R):
                        rr = r0 + j // 2
                        cs = j % 2
                        src = _ap(
                            ysb[:], yoff(rr + slot0) + cs * 128, [[1, 128]]
                        )
                        nc.tensor.transpose(
                            pts2[:, j * 128 : (j + 1) * 128], src, id128sb[:]
                        )
                    sp = stripp.tile([128, 2 * R, 256], f16, tag="strip")
                    nc.scalar.copy(
                        out=_ap(sp[:], 0, [[256, 2 * R], [1, 128]]),
                        in_=pts2[:],
                    )
                    nc.vector.tensor_tensor(
                        _ap(sp[:], 128, [[256, 2 * R], [1, 128]]),
                        _ap(sp[:], 0, [[256, 2 * R], [1, 128]]),
                        _ap(sp[:], 0, [[256, 2 * R], [1, 128]]),
                        ALU.mult,
                    )
                    for j in range(2 * R):
                        ci = ci_count
                        ci_count += 1
                        nc.tensor.matmul(
                            stats[:],
                            _ap(maskpm[:], ci * 18, [[1, 18]]),
                            sp[:, j, :],
                            start=(ci == 0), stop=(ci == NST - 1),
                        )

                # ================= stats finalize =================
                ssb = smallp.tile([18, 256], f32, tag="ssb")
                nc.scalar.copy(out=ssb[:], in_=stats[:])
                if KDEBUG:
                    nc.sync.dma_start(out=dbg_ssb[L], in_=ssb[:])
                tmp = smallp.tile([9, 128], f32, tag="tmp")
                nc.sync.dma_start(
                    out=tmp[:],
                    in_=_ap(ssb[9:18, :], 64, [[128, 2], [1, 64]]),
                )
                s1 = smallp.tile([9, 64], f32, tag="s1")
                s2 = smallp.tile([9, 64], f32, tag="s2")
                nc.vector.tensor_tensor(s1[:], ssb[0:9, 0:64], tmp[:, 0:64], ALU.add)
                nc.vector.tensor_tensor(s2[:], ssb[0:9, 128:192], tmp[:, 64:128], ALU.add)
                mean = smallp.tile([9, 64], f32, tag="mean")
                nc.vector.tensor_scalar_mul(out=mean[:], in0=s1[:], scalar1=rcsb[:])
                e2 = smallp.tile([9, 64], f32, tag="e2")
                nc.vector.tensor_scalar_mul(out=e2[:], in0=s2[:], scalar1=rcsb[:])
                var = smallp.tile([9, 64], f32, tag="var")
                nc.vector.tensor_tensor(var[:], mean[:], mean[:], ALU.mult)
                nc.vector.tensor_tensor(var[:], e2[:], var[:], ALU.subtract)
                sd = smallp.tile([9, 64], f32, tag="sd")
                nc.scalar.activation(
                    out=sd[:], in_=var[:], func=mybir.ActivationFunctionType.Sqrt,
                    bias=epsap[:], scale=1.0,
                )
                rstd = smallp.tile([9, 64], f32, tag="rstd")
                nc.vector.reciprocal(out=rstd[:], in_=sd[:])
                # ab: A at partitions 0:9, B at partitions 32:41
                ab = smallp.tile([64, 64], f32, tag="ab")
                nc.vector.memset(ab[:], 0.0)
                nc.vector.tensor_tensor(ab[0:9, :], rstd[:], gam[L][:], ALU.mult)
                mA = smallp.tile([9, 64], f32, tag="mA")
                nc.vector.tensor_tensor(mA[:], mean[:], ab[0:9, :], ALU.mult)
                nc.vector.tensor_tensor(ab[32:41, :], bet[L][:], mA[:], ALU.subtract)
                # background row: A=1, B=0
                nc.sync.dma_start(out=ab[8:9, :], in_=ones1[:])
                nc.sync.dma_start(out=ab[40:41, :], in_=zeros1[:])
                # fp16 copies of A (rows 0:9) and B (rows 32:41), then place
                # into expansion lhsT tiles [18, 128] (block-diagonal per half)
                af16 = smallp.tile([9, 64], f16, tag="af16")
                bf16t = smallp.tile([41, 64], f16, tag="bf16t")
                nc.vector.tensor_copy(af16[:], ab[0:9, :])
                nc.vector.tensor_copy(bf16t[32:41, :], ab[32:41, :])
                ab2s = smallp.tile([18, 128], f16, tag="ab2s")
                ab2o = smallp.tile([18, 128], f16, tag="ab2o")
                nc.vector.memset(ab2s[:], 0.0)
                nc.vector.memset(ab2o[:], 0.0)
                nc.sync.dma_start(out=ab2s[0:9, 0:64], in_=af16[:])
                nc.sync.dma_start(out=ab2s[9:18, 64:128], in_=af16[:])
                nc.sync.dma_start(out=ab2o[0:9, 0:64], in_=bf16t[32:41, :])
                nc.sync.dma_start(out=ab2o[9:18, 64:128], in_=bf16t[32:41, :])

                if KDEBUG and L == 0:
                    dcp = const.tile([18, 128], f32, name="dcpA")
                    nc.vector.tensor_copy(dcp[:], ab2s[:])
                    nc.sync.dma_start(out=dbg_tab[0], in_=dcp[:])
                    dcp2 = const.tile([18, 128], f32, name="dcpB")
                    nc.vector.tensor_copy(dcp2[:], ab2o[:])
                    nc.sync.dma_start(out=dbg_tab[1], in_=dcp2[:])
                    dmk = const.tile([128, NST * 18], f32, name="dmk")
                    nc.vector.tensor_copy(dmk[:], maskpm[:])
                    nc.sync.dma_start(out=dbg_mask[:].rearrange("a b c -> a (b c)"), in_=dmk[:])

                if KDEBUG and L == 0:
                    _dbg_dump(nc, ysb, dbg_y1, NCH, HH, "d1", outp)
                # ================= normalize =================
                for g in range(HH // 2):
                    base = yoff(2 * g + slot0)
                    sE = pse.tile([128, 512], f32, tag="exp", name=f"se{L}_{g}")
                    oE = pse.tile([128, 512], f32, tag="exp", name=f"oe{L}_{g}")
                    win = ms2[:, 2 * g * W : (2 * g + 2) * W]
                    nc.tensor.matmul(sE[:], ab2s[:], win, start=True, stop=True)
                    nc.tensor.matmul(oE[:], ab2o[:], win, start=True, stop=True)
                    yv = _ap(ysb[:], base, [[PITCH, 2], [1, W]])
                    t1 = normp.tile([128, 512], f16, tag="t1")
                    nc.vector.tensor_tensor(t1[:], yv, sE[:], ALU.mult)
                    nc.vector.tensor_tensor(t1[:], t1[:], oE[:], ALU.add)
                    if L == 0:
                        dst_relu = yv
                    else:
                        st = outp.tile([128, 512], f32, tag="st")
                        dst_relu = st[:]
                    if g % 2 == 0:
                        nc.vector.tensor_scalar_max(out=dst_relu, in0=t1[:], scalar1=0.0)
                    else:
                        nc.scalar.activation(
                            out=dst_relu, in_=t1[:],
                            func=mybir.ActivationFunctionType.Relu,
                        )
                    if L == 1:
                        nc.sync.dma_start(
                            out=bass.AP(
                                tensor=out[:].tensor,
                                offset=2 * g * W,
                                ap=[[HH * W, 2], [H * W, 64], [W, 2], [1, W]],
                            ),
                            in_=st[:],
                        )
                if KDEBUG and L == 0:
                    _dbg_dump(nc, ysb, dbg_y1n, NCH, HH, "d2", outp)
                if L == 0:
                    # halo rows for conv2: A slot HH+1 <- B row HH (slot 1);
                    # B slot 0 <- A row HH-1 (slot HH)
                    nc.sync.dma_start(
                        out=_ap(ysb[0:64, :], yoff(HH + 1), [[1, W]]),
                        in_=_ap(ysb[64:128, :], yoff(1), [[1, W]]),
                    )
                    nc.sync.dma_start(
                        out=_ap(ysb[64:128, :], yoff(0), [[1, W]]),
                        in_=_ap(ysb[0:64, :], yoff(HH), [[1, W]]),
                    )

    return nc


MAXW = 1


def _split_multi_waits(nc):
    """The installed walrus rejects instructions with >MAXW sync waits; hoist
    excess waits onto preceding same-engine nops."""
    nsplit = 0
    for fn in nc.m.functions:
        for blk in fn.blocks:
            insts = list(blk.instructions)
            out = []
            for inst in insts:
                si = inst.sync_info
                waits = list(si.on_wait) if (si and si.on_wait) else []
                if len(waits) > MAXW:
                    for i in range(0, len(waits) - MAXW, MAXW):
                        nop = mybir.InstNoOp(
                            name=f"WSPLIT-{nsplit}", ins=[], outs=[]
                        )
                        nsplit += 1
                        nop.engine = inst.engine
                        nop.sync_info = mybir.SyncInfo(
                            on_wait=waits[i : i + MAXW], on_update=[]
                        )
                        out.append(nop)
                    si.on_wait = waits[len(waits) - MAXW :]
                out.append(inst)
            if len(out) != len(insts):
                while len(blk.instructions):
                    blk.instructions.pop()
                for inst in out:
                    blk.instructions.append(inst)
    return nsplit


def build_nc(H=256, split_waits=True):
    _install_tile_patch()
    nc = bass.Bass()
    emit(nc, H)
    if split_waits:
        n = _split_multi_waits(nc)
        if n:
            print(f"kernel: split {n} multi-wait instructions")
    return nc


# ---------------------------------------------------------------------------
# host-side input prep
# ---------------------------------------------------------------------------
def prep_core_inputs(x_img, ids_img, w0, g0v, b0v, w1, g1v, b1v, H=256):
    """x_img [C,H,W] f32, ids_img [H,W] int -> input map for one core."""
    HH = H // 2
    NCH = HH // 4
    seg = np.where(ids_img < 0, 8, ids_img).astype(np.int64)

    m = {}
    m["xh"] = np.ascontiguousarray(x_img.reshape(C, H * W).astype(np.float16))
    m["idsf"] = np.ascontiguousarray(ids_img.reshape(H * W).astype(np.float16))
    cnt = np.bincount(seg.reshape(-1), minlength=9)[:9]
    m["rcnt"] = (1.0 / np.maximum(cnt, 1)).astype(np.float32)

    for name, wmat in (("w0d", w0), ("w1d", w1)):
        wd = np.zeros((9, 128, 128), np.float16)
        for t in range(9):
            dy, dx = t // 3, t % 3
            lhsT = wmat[:, :, dy, dx].T.astype(np.float16)  # [cin, cout]
            wd[t, 0:64, 0:64] = lhsT
            wd[t, 64:128, 64:128] = lhsT
        m[name] = np.ascontiguousarray(wd.transpose(1, 0, 2))  # [ci, t, co]

    m["id128"] = np.eye(128, dtype=np.float16)
    m["g0"] = np.asarray(g0v, np.float32)
    m["b0"] = np.asarray(b0v, np.float32)
    m["g1"] = np.asarray(g1v, np.float32)
    m["b1"] = np.asarray(b1v, np.float32)
    return m


LAST_RESULT = None


def kernel(features, ins_indices_batch, w0, g0, b0, w1, g1, b1):
    global LAST_RESULT
    _install_ntff_shim()
    from concourse.bass_utils import run_bass_kernel_spmd
    from concourse import bass2jax as _b2j
    import traceback as _tb

    _b2j.install_neuronx_cc_hook()
    import libneuronxla as _lnx

    if not getattr(_lnx, "_ant_dbg_wrapped", False):
        _orig = _lnx.neuronx_cc

        def _dbg(*a, **k):
            try:
                return _orig(*a, **k)
            except BaseException:
                _tb.print_exc()
                raise

        _lnx.neuronx_cc = _dbg
        _lnx._ant_dbg_wrapped = True

    x = np.asarray(features, np.float32)
    ids = np.asarray(ins_indices_batch).astype(np.int64)
    w0 = np.asarray(w0, np.float32)
    w1 = np.asarray(w1, np.float32)
    N = x.shape[0]
    H = x.shape[2]

    nc = build_nc(H)
    in_maps = [
        prep_core_inputs(x[i], ids[i], w0, g0, b0, w1, g1, b1, H) for i in range(N)
    ]
    trace = bool(int(os.environ.get("BASS_KERNEL_TRACE", "0")))
    res = run_bass_kernel_spmd(nc, in_maps, list(range(N)), trace=trace)
    LAST_RESULT = res
    outs = [res.results[i]["out"].reshape(C, H, W) for i in range(N)]
    return np.stack(outs, 0)

